# revision 2
# baseline (speedup 1.0000x reference)
"""Causal MHA block for Trainium2, 8 cores as (batch, head-group), fp8-accelerated.

Core c -> batch c//2, heads (c%2)*4..+4. Per core, per head:
  QKV projection: n/q rows 0-511 in bf16 (accuracy-critical low-entropy region),
  rows 512+ via fp8e4 DoubleRow matmuls (256-deep contraction, 0.5 cyc/col).
  Attention processed in kt-PAIRS (two 128-k tiles per unit):
    qb==0 (q rows 0-511): bf16 path, causally pruned ranges.
    qb>=1: fp8 DoubleRow S (K-pair stationary, Q-pair moving), single paired
      exp activation (bias -1 to keep exp < fp8e4 max), fp8 P pairs,
      DoubleRow PV and ones-DoubleRow row-sum L from the SAME quantized P
      (softmax normalization consistency).
  Out projection: rows <512 bf16, rest fp8 DoubleRow; bias on even cores.
Host sums the two per-batch partials to unshard.
"""

from collections import deque

import numpy as np
import ml_dtypes

import concourse.bass as bass
import concourse.tile as tile
from concourse import bacc, mybir
from concourse.bass_utils import run_bass_kernel_spmd

B, N, C, H = 4, 2048, 256, 8
SCALE = C ** -0.5
BF16 = ml_dtypes.bfloat16
E4M3 = ml_dtypes.float8_e4m3
FP32 = mybir.dt.float32
BF = mybir.dt.bfloat16
FP8 = mybir.dt.float8e4
DR = mybir.MatmulPerfMode.DoubleRow
HPC = 4  # heads per core


def _emit(tc, nq, aps):
    nc = tc.nc
    nb = nq // 512
    nt = nq // 128

    (xt_d, xt8_d, wqkv_d, wqkv8_d, wproj_d, wproj8_d, bias_d, ident_d,
     uneg_d, out_d) = aps
    xt_r = xt_d.rearrange("(c p) n -> p c n", p=128)
    xt8_r = xt8_d.rearrange("(c p) n -> p c n", p=128)
    wqkv_r = wqkv_d.rearrange("(c p) m -> p c m", p=128)
    wqkv8_r = wqkv8_d.rearrange("(c p) m -> p c m", p=128)
    wproj_r = wproj_d.rearrange("(t p) f -> p t f", p=128)
    wproj8_r = wproj8_d.rearrange("(t p) f -> p t f", p=128)
    out_r = out_d.rearrange("(t p) f -> p t f", p=128)

    singles = tc._es.enter_context(tc.tile_pool(name="singles", bufs=1))
    pool_h = tc._es.enter_context(tc.tile_pool(name="headp", bufs=2))
    pool_p8 = tc._es.enter_context(tc.tile_pool(name="p8p", bufs=6))
    pool_pb = tc._es.enter_context(tc.tile_pool(name="pbp", bufs=2))
    pool_misc = tc._es.enter_context(tc.tile_pool(name="miscp", bufs=2))
    pool_osb = tc._es.enter_context(tc.tile_pool(name="osbp", bufs=4))
    pool_sp = tc._es.enter_context(tc.tile_pool(name="psumsp", bufs=2, space="PSUM"))
    pool_ot = tc._es.enter_context(tc.tile_pool(name="psumot", bufs=1, space="PSUM"))
    pool_l = tc._es.enter_context(tc.tile_pool(name="psuml", bufs=1, space="PSUM"))
    pool_prj = tc._es.enter_context(tc.tile_pool(name="psumprj", bufs=1, space="PSUM"))

    # --- singles ---
    xt_sb = singles.tile([128, 2, nq], BF)
    xt8_sb = singles.tile([128, 2, nq], FP8)
    wqkv_sb = singles.tile([128, 2, 3 * HPC * C], BF)
    wqkv8_sb = singles.tile([128, 2, 3 * HPC * C], FP8)
    wproj_sb = singles.tile([128, 2 * HPC, C], BF)
    wproj8_sb = singles.tile([128, 2 * HPC, C], FP8)
    bias_sb = singles.tile([1, C], FP32)
    biasb_sb = singles.tile([128, C], FP32)
    ident_sb = singles.tile([128, 128], BF)
    uneg_sb = singles.tile([128, 2, 256], BF)
    ones_sb = singles.tile([128, 128], BF)
    ones8_sb = singles.tile([128, 2, 128], FP8)
    nbias_sb = singles.tile([128, 1], FP32)
    ot_sb = singles.tile([128, 2 * HPC, 512], BF)
    ot8_sb = singles.tile([128, 2 * HPC, nq - 512], FP8)
    p8A = singles.tile([128, 2, 512], FP8)
    p8B = singles.tile([128, 2, 512], FP8)
    warm_sb = singles.tile([128, 512], BF)

    # memsets first: no input deps, unblocks the PE warmup immediately
    nc.gpsimd.memset(warm_sb[:], 0.0)
    nc.gpsimd.memset(ones_sb[:], 1.0)
    nc.gpsimd.memset(ones8_sb[:], 1.0)
    nc.gpsimd.memset(nbias_sb[:], -1.0)
    nc.gpsimd.memset(p8A[:], 0.0)
    nc.gpsimd.memset(p8B[:], 0.0)

    # --- input DMAs: keep the scalar (ACT) queue SHORT — it must be free
    # for the first exp activations. Everything non-critical goes on sync;
    # proj weights are issued later (see attention(1) kickoff below).
    for hw in range(HPC):
        c0 = hw * 3 * C
        nc.scalar.dma_start(wqkv_sb[:, :, c0:c0 + 3 * C], wqkv_r[:, :, c0:c0 + 3 * C])
    for ib in range(nb):
        nc.sync.dma_start(xt_sb[:, :, ib * 512:(ib + 1) * 512],
                          xt_r[:, :, ib * 512:(ib + 1) * 512])
    nc.sync.dma_start(ident_sb[:], ident_d[:])
    nc.sync.dma_start(uneg_sb[:], uneg_d[:])
    for hw in range(2):
        nc.sync.dma_start(xt8_sb[:, :, hw * 1024:(hw + 1) * 1024],
                          xt8_r[:, :, hw * 1024:(hw + 1) * 1024])
        nc.sync.dma_start(wqkv8_sb[:, :, hw * 6 * C:(hw + 1) * 6 * C],
                          wqkv8_r[:, :, hw * 6 * C:(hw + 1) * 6 * C])

    def late_dmas():
        nc.sync.dma_start(wproj_sb[:], wproj_r)
        nc.sync.dma_start(wproj8_sb[:], wproj8_r)
        nc.sync.dma_start(bias_sb[:], bias_d[:])
        nc.gpsimd.partition_broadcast(biasb_sb[:], bias_sb[:])

    # warm up the PE clock gate while input DMAs land
    warm_ps = pool_prj.tile([128, 512], FP32, tag="prj", name="warm_ps")
    for wi in range(12):
        nc.tensor.matmul(warm_ps[:], warm_sb[:, :128], warm_sb[:],
                         start=(wi == 0), stop=(wi == 11))

    sp_ring = [pool_sp.tile([128, 2, 512], FP32, tag="sp", name=f"spring{i}")
               for i in range(2)]
    ring = {"i": 0}

    def sp_tile():
        t = sp_ring[ring["i"] % 2]
        ring["i"] += 1
        return t

    def alloc_head_tiles():
        qt_sb = pool_h.tile([128, 2, 512], BF, tag="qt", name="qt")
        qt8_sb = pool_h.tile([128, 2, nq - 512], FP8, tag="qt8", name="qt8")
        kt_sb = pool_h.tile([128, 2, 512], BF, tag="kt", name="kt")
        kt8_sb = pool_h.tile([128, 2, nq], FP8, tag="kt8", name="kt8")
        v_sb = pool_h.tile([128, 4, C], BF, tag="v", name="v")
        v8_sb = pool_h.tile([128, nt, C], FP8, tag="v8", name="v8")
        return qt_sb, qt8_sb, kt_sb, kt8_sb, v_sb, v8_sb

    def qkv_units(hp, tiles):
        """Pair-granular projection units for head hp. Ordered so that the
        k-th attention q-block of this head only needs units consumed by
        early drip slots."""
        qt_sb, qt8_sb, kt_sb, kt8_sb, v_sb, v8_sb = tiles

        def u_qk_bf(j):
            # n/q cols 0-511, both head-dim halves; bf16 (+ fp8 dup for K)
            def go():
                ps = sp_tile()
                for ct in range(2):
                    col0 = hp * 3 * C + j * C + ct * 128
                    for ci in range(2):
                        nc.tensor.matmul(
                            ps[:, ct, :], wqkv_sb[:, ci, col0:col0 + 128],
                            xt_sb[:, ci, 0:512], start=(ci == 0), stop=(ci == 1))
                dst = qt_sb if j == 0 else kt_sb
                nc.vector.tensor_copy(dst[:, :, :], ps[:])
                if j == 1:
                    nc.gpsimd.tensor_copy(kt8_sb[:, :, 0:512], dst[:, :, :])
            return go

        def u_qk_f8(j, ib):
            def go():
                ps = sp_tile()
                for ct in range(2):
                    col0 = hp * 3 * C + j * C + ct * 128
                    nc.tensor.matmul(
                        ps[:, ct, :], wqkv8_sb[:, :, col0:col0 + 128],
                        xt8_sb[:, :, ib * 512:(ib + 1) * 512],
                        start=True, stop=True, perf_mode=DR)
                if j == 0:
                    nc.vector.tensor_copy(
                        qt8_sb[:, :, (ib - 1) * 512:ib * 512], ps[:])
                else:
                    nc.scalar.copy(
                        kt8_sb[:, :, ib * 512:(ib + 1) * 512], ps[:])
            return go

        def u_v_bf(itp):
            def go():
                ps = sp_tile()
                vcol = hp * 3 * C + 2 * C
                for s in range(2):
                    it = 2 * itp + s
                    for ci in range(2):
                        nc.tensor.matmul(
                            ps[:, s, 0:C], xt_sb[:, ci, it * 128:(it + 1) * 128],
                            wqkv_sb[:, ci, vcol:vcol + C],
                            start=(ci == 0), stop=(ci == 1))
                nc.vector.tensor_copy(v_sb[:, 2 * itp:2 * itp + 2, :], ps[:, :, 0:C])
                nc.gpsimd.tensor_copy(v8_sb[:, 2 * itp:2 * itp + 2, :],
                                      v_sb[:, 2 * itp:2 * itp + 2, :])
            return go

        def u_v_f8(itp):
            def go():
                ps = sp_tile()
                vcol = hp * 3 * C + 2 * C
                for s in range(2):
                    it = 2 * itp + s
                    nc.tensor.matmul(
                        ps[:, s, 0:C], xt8_sb[:, :, it * 128:(it + 1) * 128],
                        wqkv8_sb[:, :, vcol:vcol + C],
                        start=True, stop=True, perf_mode=DR)
                nc.vector.tensor_copy(v8_sb[:, 2 * itp:2 * itp + 2, :], ps[:, :, 0:C])
            return go

        units = [u_qk_bf(0), u_qk_bf(1), u_v_bf(0), u_v_bf(1)]
        for ib in range(1, nb):
            units.append(u_qk_f8(1, ib))
            units.append(u_qk_f8(0, ib))
            units.append(u_v_f8(2 * ib))
            units.append(u_v_f8(2 * ib + 1))
        return units

    def proj_unit(it, fast=False):
        def go():
            if fast:
                ps = sp_tile()[:, 0, :]
            else:
                ps = pool_prj.tile([128, 512], FP32, tag="prj", name="prj_ps")
            if it < 4:
                for t in range(2 * HPC):
                    nc.tensor.matmul(
                        ps[:, :C], ot_sb[:, t, it * 128:(it + 1) * 128],
                        wproj_sb[:, t, :], start=(t == 0), stop=(t == 2 * HPC - 1))
            else:
                for tp in range(HPC):
                    nc.tensor.matmul(
                        ps[:, :C],
                        ot8_sb[:, 2 * tp:2 * tp + 2, (it - 4) * 128:(it - 3) * 128],
                        wproj8_sb[:, 2 * tp:2 * tp + 2, :],
                        start=(tp == 0), stop=(tp == HPC - 1), perf_mode=DR)
            osb = pool_osb.tile([128, C], FP32, tag="osb", name="osb")
            nc.vector.tensor_tensor(osb[:], ps[:, :C], biasb_sb[:], mybir.AluOpType.add)
            eng = nc.sync if it % 2 == 0 else nc.scalar
            eng.dma_start(out_r[:, it, :], osb[:])
        return go

    def attention(hp, tiles, drip):
        qt_sb, qt8_sb, kt_sb, kt8_sb, v_sb, v8_sb = tiles

        steps = []
        for qb in range(nb):
            npr = 2 * qb + 2
            for pr in range(npr):
                steps.append((qb, pr, pr == npr - 1))

        state = {}

        def emit_S(step):
            qb, pr, last = step
            if pr == 0:
                state[qb] = (
                    pool_ot.tile([128, 2, 512], FP32, tag="otp", name="otp"),
                    pool_l.tile([128, 512], FP32, tag="l", name="lp"),
                )
            s_ps = sp_tile()
            if qb == 0:
                p_t = pool_pb.tile([128, 2, 512], BF, tag="pb", name="pb")
                c0 = 2 * pr * 128
                for s in range(2):
                    kti = 2 * pr + s
                    q0 = kti * 128
                    for ci in range(2):
                        nc.tensor.matmul(
                            s_ps[:, s, q0:512], kt_sb[:, ci, kti * 128:(kti + 1) * 128],
                            qt_sb[:, ci, q0:512], start=(ci == 0), stop=False,
                            skip_group_check=True)
                nc.tensor.matmul(
                    s_ps[:, :, c0:c0 + 256], ident_sb[:], uneg_sb[:],
                    start=False, stop=True, skip_group_check=True)
                for s in range(2):
                    kti = 2 * pr + s
                    q0 = kti * 128
                    nc.scalar.activation(
                        p_t[:, s, q0:512], s_ps[:, s, q0:512],
                        mybir.ActivationFunctionType.Exp, scale=SCALE)
            else:
                diag = pr >= 2 * qb
                if diag:
                    p_t = p8A if pr == 2 * qb else p8B
                else:
                    p_t = pool_p8.tile([128, 2, 512], FP8, tag="p8", name="p8")
                d0 = 2 * (pr - 2 * qb)
                for s in range(2):
                    kti = 2 * pr + s
                    q0 = (d0 + s) * 128 if diag else 0
                    nc.tensor.matmul(
                        s_ps[:, s, q0:512], kt8_sb[:, :, kti * 128:(kti + 1) * 128],
                        qt8_sb[:, :, (qb - 1) * 512 + q0:qb * 512],
                        start=True, stop=not diag, perf_mode=DR,
                        skip_group_check=diag)
                if diag:
                    c0 = d0 * 128
                    nc.tensor.matmul(
                        s_ps[:, :, c0:c0 + 256], ident_sb[:], uneg_sb[:],
                        start=False, stop=True, skip_group_check=True)
                if not diag:
                    nc.scalar.activation(
                        p_t[:], s_ps[:], mybir.ActivationFunctionType.Exp,
                        bias=nbias_sb[:], scale=SCALE)
                else:
                    d0 = 2 * (pr - 2 * qb)
                    lo = (d0 + 1) * 128
                    nc.scalar.activation(
                        p_t[:, :, lo:512], s_ps[:, :, lo:512],
                        mybir.ActivationFunctionType.Exp,
                        bias=nbias_sb[:], scale=SCALE)
                    nc.scalar.activation(
                        p_t[:, 0, d0 * 128:lo], s_ps[:, 0, d0 * 128:lo],
                        mybir.ActivationFunctionType.Exp,
                        bias=nbias_sb[:], scale=SCALE)
            return p_t

        def emit_PVL(step, p_t):
            qb, pr, last = step
            otp, lps = state[qb]
            first = pr == 0
            if qb == 0:
                for s in range(2):
                    kti = 2 * pr + s
                    q0 = kti * 128
                    for h2 in range(2):
                        nc.tensor.matmul(
                            otp[:, h2, q0:512], v_sb[:, kti, h2 * 128:(h2 + 1) * 128],
                            p_t[:, s, q0:512],
                            start=(first and s == 0), stop=(last and s == 1))
                    nc.tensor.matmul(
                        lps[:, q0:512], ones_sb[:], p_t[:, s, q0:512],
                        start=(first and s == 0), stop=(last and s == 1))
            else:
                # diag pair B (d0=2) only has causal columns >= 256
                pv0 = 256 if pr == 2 * qb + 1 else 0
                for h2 in range(2):
                    nc.tensor.matmul(
                        otp[:, h2, pv0:], v8_sb[:, 2 * pr:2 * pr + 2, h2 * 128:(h2 + 1) * 128],
                        p_t[:, :, pv0:], start=first, stop=last, perf_mode=DR)
                nc.tensor.matmul(
                    lps[:, pv0:], ones8_sb[:], p_t[:, :, pv0:],
                    start=first, stop=last, perf_mode=DR)
            if last:
                finalize(qb)

        def finalize(qb):
            otp, lps = state[qb]
            rb = pool_misc.tile([128, 512], FP32, tag="rb", name="rb")
            nc.vector.reciprocal_approx_fast(rb[:], lps[:])
            for h2 in range(2):
                if qb == 0:
                    nc.vector.tensor_tensor(
                        ot_sb[:, hp * 2 + h2, :], otp[:, h2, :], rb[:],
                        mybir.AluOpType.mult)
                else:
                    nc.vector.tensor_tensor(
                        ot8_sb[:, hp * 2 + h2, (qb - 1) * 512:qb * 512],
                        otp[:, h2, :], rb[:], mybir.AluOpType.mult)
            if hp == HPC - 1:
                for it in range(4 * qb, 4 * qb + 4):
                    drip.append(proj_unit(it, fast=(qb == nb - 1)))

        work = []
        for i, step in enumerate(steps):
            work.append((step, emit_S(step)))
            if i >= 2:
                emit_PVL(*work[i - 2])
            if drip:
                drip.popleft()()
        emit_PVL(*work[-2])
        emit_PVL(*work[-1])

    head_tiles = alloc_head_tiles()
    units0 = qkv_units(0, head_tiles)
    for u in units0[:4]:
        u()
    drip = deque(units0[4:])
    for hp in range(HPC):
        if hp == 1:
            late_dmas()
        if hp + 1 < HPC:
            nxt_tiles = alloc_head_tiles()
            drip.extend(qkv_units(hp + 1, nxt_tiles))
        else:
            nxt_tiles = None
        attention(hp, head_tiles, drip)
        head_tiles = nxt_tiles
    while drip:
        drip.popleft()()


def build_program(nq=N):
    nc = bacc.Bacc(trn_type="TRN2")
    xt_d = nc.dram_tensor("xt", (C, nq), BF, kind="ExternalInput").ap()
    xt8_d = nc.dram_tensor("xt8", (C, nq), FP8, kind="ExternalInput").ap()
    wqkv_d = nc.dram_tensor("wqkv", (C, 3 * HPC * C), BF, kind="ExternalInput").ap()
    wqkv8_d = nc.dram_tensor("wqkv8", (C, 3 * HPC * C), FP8, kind="ExternalInput").ap()
    wproj_d = nc.dram_tensor("wproj", (2 * HPC * 128, C), BF, kind="ExternalInput").ap()
    wproj8_d = nc.dram_tensor("wproj8", (2 * HPC * 128, C), FP8, kind="ExternalInput").ap()
    bias_d = nc.dram_tensor("bias", (1, C), FP32, kind="ExternalInput").ap()
    ident_d = nc.dram_tensor("ident", (128, 128), BF, kind="ExternalInput").ap()
    uneg_d = nc.dram_tensor("uneg", (128, 2, 256), BF, kind="ExternalInput").ap()
    out_d = nc.dram_tensor("out", (nq, C), FP32, kind="ExternalOutput").ap()
    with tile.TileContext(nc) as tc:
        import contextlib
        tc._es = contextlib.ExitStack()
        with tc._es:
            _emit(tc, nq, (xt_d, xt8_d, wqkv_d, wqkv8_d, wproj_d, wproj8_d,
                           bias_d, ident_d, uneg_d, out_d))
    nc.compile()
    return nc


def core_inputs(core, x, w_qkv, w_proj, b_proj, nq=N):
    b, hg = core // 2, core % 2
    heads = list(range(hg * HPC, hg * HPC + HPC))
    xt32 = np.ascontiguousarray(x[b].T)
    xt = xt32.astype(BF16)
    xt8 = xt32.astype(E4M3)
    wr = np.asarray(w_qkv, np.float32).reshape(C, 3, H, C)
    w4 = np.ascontiguousarray(
        wr[:, :, heads, :].transpose(0, 2, 1, 3)
    ).reshape(C, 3 * HPC * C)
    wp = np.asarray(w_proj, np.float32).reshape(H, C, C)[heads].reshape(HPC * C, C)
    bias = (np.asarray(b_proj, np.float32) if hg == 0
            else np.zeros(C, np.float32)).reshape(1, C)
    ident = np.eye(128, dtype=np.float32).astype(BF16)
    tri = np.where(np.arange(128)[:, None] > np.arange(128)[None, :],
                   np.float32(-30000.0), np.float32(0.0))
    uneg = np.zeros((128, 2, 256), np.float32)
    uneg[:, 0, 0:128] = tri                 # slab0: tri in first 128 cols
    uneg[:, 1, 0:128] = -30000.0            # slab1: fully masked region
    uneg[:, 1, 128:256] = tri               # slab1: tri in second 128 cols
    uneg = uneg.astype(BF16)
    return {"xt": xt, "xt8": xt8,
            "wqkv": w4.astype(BF16), "wqkv8": w4.astype(E4M3),
            "wproj": wp.astype(BF16), "wproj8": wp.astype(E4M3),
            "bias": bias, "ident": ident, "uneg": uneg}


_CACHE = {}


def kernel(x, w_qkv, w_proj, b_proj, **run_kwargs):
    x = np.asarray(x, np.float32)
    w_qkv = np.asarray(w_qkv, np.float32)
    w_proj = np.asarray(w_proj, np.float32)
    b_proj = np.asarray(b_proj, np.float32)
    if "nc" not in _CACHE:
        _CACHE["nc"] = build_program(N)
    nc = _CACHE["nc"]
    in_maps = [core_inputs(c, x, w_qkv, w_proj, b_proj) for c in range(8)]
    res = run_bass_kernel_spmd(nc, in_maps, core_ids=list(range(8)), **run_kwargs)
    out = np.zeros((B, N, C), np.float32)
    for c in range(8):
        out[c // 2] += res.results[c]["out"]
    _CACHE["last_results"] = res
    return out


# revision 3
# speedup vs baseline: 1.0178x; 1.0178x over previous
"""Causal MHA block for Trainium2, 8 cores as (batch, head-group), fp8-accelerated.

Core c -> batch c//2, heads (c%2)*4..+4. Per core, per head:
  QKV projection: n/q rows 0-511 in bf16 (accuracy-critical low-entropy region),
  rows 512+ via fp8e4 DoubleRow matmuls (256-deep contraction, 0.5 cyc/col).
  Attention processed in kt-PAIRS (two 128-k tiles per unit):
    qb==0 (q rows 0-511): bf16 path, causally pruned ranges.
    qb>=1: fp8 DoubleRow S (K-pair stationary, Q-pair moving), single paired
      exp activation (bias -1 to keep exp < fp8e4 max), fp8 P pairs,
      DoubleRow PV and ones-DoubleRow row-sum L from the SAME quantized P
      (softmax normalization consistency).
  Out projection: rows <512 bf16, rest fp8 DoubleRow; bias on even cores.
Host sums the two per-batch partials to unshard.
"""

from collections import deque

import numpy as np
import ml_dtypes

import concourse.bass as bass
import concourse.tile as tile
from concourse import bacc, mybir
from concourse.bass_utils import run_bass_kernel_spmd

B, N, C, H = 4, 2048, 256, 8
SCALE = C ** -0.5
BF16 = ml_dtypes.bfloat16
E4M3 = ml_dtypes.float8_e4m3
FP32 = mybir.dt.float32
BF = mybir.dt.bfloat16
FP8 = mybir.dt.float8e4
DR = mybir.MatmulPerfMode.DoubleRow
HPC = 4  # heads per core


def _emit(tc, nq, aps):
    nc = tc.nc
    nb = nq // 512
    nt = nq // 128

    (xt_d, xt8_d, wqkv_d, wqkv8_d, wproj_d, wproj8_d, bias_d, ident_d,
     uneg_d, out_d) = aps
    xt_r = xt_d.rearrange("(c p) n -> p c n", p=128)
    xt8_r = xt8_d.rearrange("(c p) n -> p c n", p=128)
    wqkv_r = wqkv_d.rearrange("(c p) m -> p c m", p=128)
    wqkv8_r = wqkv8_d.rearrange("(c p) m -> p c m", p=128)
    wproj_r = wproj_d.rearrange("(t p) f -> p t f", p=128)
    wproj8_r = wproj8_d.rearrange("(t p) f -> p t f", p=128)
    out_r = out_d.rearrange("(t p) f -> p t f", p=128)

    singles = tc._es.enter_context(tc.tile_pool(name="singles", bufs=1))
    pool_h = tc._es.enter_context(tc.tile_pool(name="headp", bufs=2))
    pool_p8 = tc._es.enter_context(tc.tile_pool(name="p8p", bufs=6))
    pool_pb = tc._es.enter_context(tc.tile_pool(name="pbp", bufs=2))
    pool_misc = tc._es.enter_context(tc.tile_pool(name="miscp", bufs=2))
    pool_osb = tc._es.enter_context(tc.tile_pool(name="osbp", bufs=4))
    pool_sp = tc._es.enter_context(tc.tile_pool(name="psumsp", bufs=2, space="PSUM"))
    pool_ot = tc._es.enter_context(tc.tile_pool(name="psumot", bufs=1, space="PSUM"))
    pool_l = tc._es.enter_context(tc.tile_pool(name="psuml", bufs=1, space="PSUM"))
    pool_prj = tc._es.enter_context(tc.tile_pool(name="psumprj", bufs=1, space="PSUM"))

    # --- singles ---
    xt_sb = singles.tile([128, 2, nq], BF)
    xt8_sb = singles.tile([128, 2, nq], FP8)
    wqkv_sb = singles.tile([128, 2, 3 * HPC * C], BF)
    wqkv8_sb = singles.tile([128, 2, 3 * HPC * C], FP8)
    wproj_sb = singles.tile([128, 2 * HPC, C], BF)
    wproj8_sb = singles.tile([128, 2 * HPC, C], FP8)
    bias_sb = singles.tile([1, C], FP32)
    biasb_sb = singles.tile([128, C], FP32)
    ident_sb = singles.tile([128, 128], BF)
    uneg_sb = singles.tile([128, 2, 256], BF)
    ones_sb = singles.tile([128, 128], BF)
    ones8_sb = singles.tile([128, 2, 128], FP8)
    nbias_sb = singles.tile([128, 1], FP32)
    ot_sb = singles.tile([128, 2 * HPC, 512], BF)
    ot8_sb = singles.tile([128, 2 * HPC, nq - 512], FP8)
    p8A = singles.tile([128, 2, 512], FP8)
    p8B = singles.tile([128, 2, 512], FP8)

    # memsets first: no input deps
    nc.gpsimd.memset(ones_sb[:], 1.0)
    nc.gpsimd.memset(ones8_sb[:], 1.0)
    nc.gpsimd.memset(nbias_sb[:], -1.0)
    nc.gpsimd.memset(p8A[:], 0.0)
    nc.gpsimd.memset(p8B[:], 0.0)

    # --- input DMAs: keep the scalar (ACT) queue SHORT — it must be free
    # for the first exp activations. Everything non-critical goes on sync;
    # proj weights are issued later (see attention(1) kickoff below).
    # head 0's q+k weight columns first: they gate the very first matmul
    nc.scalar.dma_start(wqkv_sb[:, :, 0:2 * C], wqkv_r[:, :, 0:2 * C])
    nc.scalar.dma_start(wqkv_sb[:, :, 2 * C:3 * C], wqkv_r[:, :, 2 * C:3 * C])
    for hw in range(1, HPC):
        c0 = hw * 3 * C
        nc.scalar.dma_start(wqkv_sb[:, :, c0:c0 + 3 * C], wqkv_r[:, :, c0:c0 + 3 * C])
    nc.sync.dma_start(xt_sb[:, :, 0:512], xt_r[:, :, 0:512])
    for ib in range(1, nb):
        nc.sync.dma_start(xt_sb[:, :, ib * 512:(ib + 1) * 512],
                          xt_r[:, :, ib * 512:(ib + 1) * 512])
    nc.sync.dma_start(ident_sb[:], ident_d[:])
    nc.sync.dma_start(uneg_sb[:], uneg_d[:])
    for hw in range(2):
        nc.sync.dma_start(xt8_sb[:, :, hw * 1024:(hw + 1) * 1024],
                          xt8_r[:, :, hw * 1024:(hw + 1) * 1024])
        nc.sync.dma_start(wqkv8_sb[:, :, hw * 6 * C:(hw + 1) * 6 * C],
                          wqkv8_r[:, :, hw * 6 * C:(hw + 1) * 6 * C])

    def late_dmas():
        nc.sync.dma_start(wproj_sb[:], wproj_r)
        nc.sync.dma_start(wproj8_sb[:], wproj8_r)
        nc.sync.dma_start(bias_sb[:], bias_d[:])
        nc.gpsimd.partition_broadcast(biasb_sb[:], bias_sb[:])

    # no separate warmup: head 0's projection matmuls serve as the PE
    # clock-gate ramp while the remaining input DMAs land

    sp_ring = [pool_sp.tile([128, 2, 512], FP32, tag="sp", name=f"spring{i}")
               for i in range(2)]
    ring = {"i": 0}

    def sp_tile():
        t = sp_ring[ring["i"] % 2]
        ring["i"] += 1
        return t

    def alloc_head_tiles():
        qt_sb = pool_h.tile([128, 2, 512], BF, tag="qt", name="qt")
        qt8_sb = pool_h.tile([128, 2, nq - 512], FP8, tag="qt8", name="qt8")
        kt_sb = pool_h.tile([128, 2, 512], BF, tag="kt", name="kt")
        kt8_sb = pool_h.tile([128, 2, nq], FP8, tag="kt8", name="kt8")
        v_sb = pool_h.tile([128, 4, C], BF, tag="v", name="v")
        v8_sb = pool_h.tile([128, nt, C], FP8, tag="v8", name="v8")
        return qt_sb, qt8_sb, kt_sb, kt8_sb, v_sb, v8_sb

    def qkv_units(hp, tiles):
        """Pair-granular projection units for head hp. Ordered so that the
        k-th attention q-block of this head only needs units consumed by
        early drip slots."""
        qt_sb, qt8_sb, kt_sb, kt8_sb, v_sb, v8_sb = tiles

        def u_qk_bf(j):
            # n/q cols 0-511, both head-dim halves; bf16 (+ fp8 dup for K)
            def go():
                ps = sp_tile()
                for ct in range(2):
                    col0 = hp * 3 * C + j * C + ct * 128
                    for ci in range(2):
                        nc.tensor.matmul(
                            ps[:, ct, :], wqkv_sb[:, ci, col0:col0 + 128],
                            xt_sb[:, ci, 0:512], start=(ci == 0), stop=(ci == 1))
                dst = qt_sb if j == 0 else kt_sb
                nc.vector.tensor_copy(dst[:, :, :], ps[:])
                if j == 1:
                    nc.gpsimd.tensor_copy(kt8_sb[:, :, 0:512], dst[:, :, :])
            return go

        def u_qk_f8(j, ib):
            def go():
                ps = sp_tile()
                for ct in range(2):
                    col0 = hp * 3 * C + j * C + ct * 128
                    nc.tensor.matmul(
                        ps[:, ct, :], wqkv8_sb[:, :, col0:col0 + 128],
                        xt8_sb[:, :, ib * 512:(ib + 1) * 512],
                        start=True, stop=True, perf_mode=DR)
                if j == 0:
                    nc.vector.tensor_copy(
                        qt8_sb[:, :, (ib - 1) * 512:ib * 512], ps[:])
                else:
                    nc.scalar.copy(
                        kt8_sb[:, :, ib * 512:(ib + 1) * 512], ps[:])
            return go

        def u_v_bf(itp):
            def go():
                ps = sp_tile()
                vcol = hp * 3 * C + 2 * C
                for s in range(2):
                    it = 2 * itp + s
                    for ci in range(2):
                        nc.tensor.matmul(
                            ps[:, s, 0:C], xt_sb[:, ci, it * 128:(it + 1) * 128],
                            wqkv_sb[:, ci, vcol:vcol + C],
                            start=(ci == 0), stop=(ci == 1))
                nc.vector.tensor_copy(v_sb[:, 2 * itp:2 * itp + 2, :], ps[:, :, 0:C])
                nc.gpsimd.tensor_copy(v8_sb[:, 2 * itp:2 * itp + 2, :],
                                      v_sb[:, 2 * itp:2 * itp + 2, :])
            return go

        def u_v_f8(itp):
            def go():
                ps = sp_tile()
                vcol = hp * 3 * C + 2 * C
                for s in range(2):
                    it = 2 * itp + s
                    nc.tensor.matmul(
                        ps[:, s, 0:C], xt8_sb[:, :, it * 128:(it + 1) * 128],
                        wqkv8_sb[:, :, vcol:vcol + C],
                        start=True, stop=True, perf_mode=DR)
                nc.vector.tensor_copy(v8_sb[:, 2 * itp:2 * itp + 2, :], ps[:, :, 0:C])
            return go

        units = [u_qk_bf(0), u_qk_bf(1), u_v_bf(0), u_v_bf(1)]
        for ib in range(1, nb):
            units.append(u_qk_f8(1, ib))
            units.append(u_qk_f8(0, ib))
            units.append(u_v_f8(2 * ib))
            units.append(u_v_f8(2 * ib + 1))
        return units

    def proj_unit(it, fast=False):
        def go():
            if fast:
                ps = sp_tile()[:, 0, :]
            else:
                ps = pool_prj.tile([128, 512], FP32, tag="prj", name="prj_ps")
            if it < 4:
                for t in range(2 * HPC):
                    nc.tensor.matmul(
                        ps[:, :C], ot_sb[:, t, it * 128:(it + 1) * 128],
                        wproj_sb[:, t, :], start=(t == 0), stop=(t == 2 * HPC - 1))
            else:
                for tp in range(HPC):
                    nc.tensor.matmul(
                        ps[:, :C],
                        ot8_sb[:, 2 * tp:2 * tp + 2, (it - 4) * 128:(it - 3) * 128],
                        wproj8_sb[:, 2 * tp:2 * tp + 2, :],
                        start=(tp == 0), stop=(tp == HPC - 1), perf_mode=DR)
            osb = pool_osb.tile([128, C], FP32, tag="osb", name="osb")
            nc.vector.tensor_tensor(osb[:], ps[:, :C], biasb_sb[:], mybir.AluOpType.add)
            eng = nc.sync if it % 2 == 0 else nc.scalar
            eng.dma_start(out_r[:, it, :], osb[:])
        return go

    pending = deque()  # PV/L closures pipelined across q-blocks AND heads

    def attention(hp, tiles, drip):
        qt_sb, qt8_sb, kt_sb, kt8_sb, v_sb, v8_sb = tiles

        steps = []
        for qb in range(nb):
            npr = 2 * qb + 2
            for pr in range(npr):
                steps.append((qb, pr, pr == npr - 1))

        state = {}

        def emit_S(step):
            qb, pr, last = step
            if pr == 0:
                state[qb] = (
                    pool_ot.tile([128, 2, 512], FP32, tag="otp", name="otp"),
                    pool_l.tile([128, 512], FP32, tag="l", name="lp"),
                )
            s_ps = sp_tile()
            if qb == 0:
                p_t = pool_pb.tile([128, 2, 512], BF, tag="pb", name="pb")
                c0 = 2 * pr * 128
                for s in range(2):
                    kti = 2 * pr + s
                    q0 = kti * 128
                    for ci in range(2):
                        nc.tensor.matmul(
                            s_ps[:, s, q0:512], kt_sb[:, ci, kti * 128:(kti + 1) * 128],
                            qt_sb[:, ci, q0:512], start=(ci == 0), stop=False,
                            skip_group_check=True)
                nc.tensor.matmul(
                    s_ps[:, :, c0:c0 + 256], ident_sb[:], uneg_sb[:],
                    start=False, stop=True, skip_group_check=True)
                for s in range(2):
                    kti = 2 * pr + s
                    q0 = kti * 128
                    nc.scalar.activation(
                        p_t[:, s, q0:512], s_ps[:, s, q0:512],
                        mybir.ActivationFunctionType.Exp, scale=SCALE)
            else:
                diag = pr >= 2 * qb
                if diag:
                    p_t = p8A if pr == 2 * qb else p8B
                else:
                    p_t = pool_p8.tile([128, 2, 512], FP8, tag="p8", name="p8")
                d0 = 2 * (pr - 2 * qb)
                for s in range(2):
                    kti = 2 * pr + s
                    q0 = (d0 + s) * 128 if diag else 0
                    nc.tensor.matmul(
                        s_ps[:, s, q0:512], kt8_sb[:, :, kti * 128:(kti + 1) * 128],
                        qt8_sb[:, :, (qb - 1) * 512 + q0:qb * 512],
                        start=True, stop=not diag, perf_mode=DR,
                        skip_group_check=diag)
                if diag:
                    c0 = d0 * 128
                    nc.tensor.matmul(
                        s_ps[:, :, c0:c0 + 256], ident_sb[:], uneg_sb[:],
                        start=False, stop=True, skip_group_check=True)
                if not diag:
                    nc.scalar.activation(
                        p_t[:], s_ps[:], mybir.ActivationFunctionType.Exp,
                        bias=nbias_sb[:], scale=SCALE)
                else:
                    d0 = 2 * (pr - 2 * qb)
                    lo = (d0 + 1) * 128
                    nc.scalar.activation(
                        p_t[:, :, lo:512], s_ps[:, :, lo:512],
                        mybir.ActivationFunctionType.Exp,
                        bias=nbias_sb[:], scale=SCALE)
                    nc.scalar.activation(
                        p_t[:, 0, d0 * 128:lo], s_ps[:, 0, d0 * 128:lo],
                        mybir.ActivationFunctionType.Exp,
                        bias=nbias_sb[:], scale=SCALE)
            return p_t

        def emit_PVL(step, p_t):
            qb, pr, last = step
            otp, lps = state[qb]
            first = pr == 0
            if qb == 0:
                for s in range(2):
                    kti = 2 * pr + s
                    q0 = kti * 128
                    for h2 in range(2):
                        nc.tensor.matmul(
                            otp[:, h2, q0:512], v_sb[:, kti, h2 * 128:(h2 + 1) * 128],
                            p_t[:, s, q0:512],
                            start=(first and s == 0), stop=(last and s == 1))
                    nc.tensor.matmul(
                        lps[:, q0:512], ones_sb[:], p_t[:, s, q0:512],
                        start=(first and s == 0), stop=(last and s == 1))
            else:
                # diag pair B (d0=2) only has causal columns >= 256
                pv0 = 256 if pr == 2 * qb + 1 else 0
                for h2 in range(2):
                    nc.tensor.matmul(
                        otp[:, h2, pv0:], v8_sb[:, 2 * pr:2 * pr + 2, h2 * 128:(h2 + 1) * 128],
                        p_t[:, :, pv0:], start=first, stop=last, perf_mode=DR)
                nc.tensor.matmul(
                    lps[:, pv0:], ones8_sb[:], p_t[:, :, pv0:],
                    start=first, stop=last, perf_mode=DR)
            if last:
                finalize(qb)

        def finalize(qb):
            otp, lps = state[qb]
            rb = pool_misc.tile([128, 512], FP32, tag="rb", name="rb")
            nc.vector.reciprocal_approx_fast(rb[:], lps[:])
            for h2 in range(2):
                if qb == 0:
                    nc.vector.tensor_tensor(
                        ot_sb[:, hp * 2 + h2, :], otp[:, h2, :], rb[:],
                        mybir.AluOpType.mult)
                else:
                    nc.vector.tensor_tensor(
                        ot8_sb[:, hp * 2 + h2, (qb - 1) * 512:qb * 512],
                        otp[:, h2, :], rb[:], mybir.AluOpType.mult)
            if hp == HPC - 1:
                for it in range(4 * qb, 4 * qb + 4):
                    drip.append(proj_unit(it, fast=(qb == nb - 1)))

        for step in steps:
            p_t = emit_S(step)
            pending.append(lambda st=step, p=p_t: emit_PVL(st, p))
            while len(pending) > 2:
                pending.popleft()()
            if drip:
                drip.popleft()()

    head_tiles = alloc_head_tiles()
    units0 = qkv_units(0, head_tiles)
    for u in units0[:4]:
        u()
    drip = deque(units0[4:])
    for hp in range(HPC):
        if hp == 1:
            late_dmas()
        if hp + 1 < HPC:
            nxt_tiles = alloc_head_tiles()
            drip.extend(qkv_units(hp + 1, nxt_tiles))
        else:
            nxt_tiles = None
        attention(hp, head_tiles, drip)
        head_tiles = nxt_tiles
    while pending:
        pending.popleft()()
    while drip:
        drip.popleft()()


def build_program(nq=N):
    nc = bacc.Bacc(trn_type="TRN2")
    xt_d = nc.dram_tensor("xt", (C, nq), BF, kind="ExternalInput").ap()
    xt8_d = nc.dram_tensor("xt8", (C, nq), FP8, kind="ExternalInput").ap()
    wqkv_d = nc.dram_tensor("wqkv", (C, 3 * HPC * C), BF, kind="ExternalInput").ap()
    wqkv8_d = nc.dram_tensor("wqkv8", (C, 3 * HPC * C), FP8, kind="ExternalInput").ap()
    wproj_d = nc.dram_tensor("wproj", (2 * HPC * 128, C), BF, kind="ExternalInput").ap()
    wproj8_d = nc.dram_tensor("wproj8", (2 * HPC * 128, C), FP8, kind="ExternalInput").ap()
    bias_d = nc.dram_tensor("bias", (1, C), FP32, kind="ExternalInput").ap()
    ident_d = nc.dram_tensor("ident", (128, 128), BF, kind="ExternalInput").ap()
    uneg_d = nc.dram_tensor("uneg", (128, 2, 256), BF, kind="ExternalInput").ap()
    out_d = nc.dram_tensor("out", (nq, C), FP32, kind="ExternalOutput").ap()
    with tile.TileContext(nc) as tc:
        import contextlib
        tc._es = contextlib.ExitStack()
        with tc._es:
            _emit(tc, nq, (xt_d, xt8_d, wqkv_d, wqkv8_d, wproj_d, wproj8_d,
                           bias_d, ident_d, uneg_d, out_d))
    nc.compile()
    return nc


def core_inputs(core, x, w_qkv, w_proj, b_proj, nq=N):
    b, hg = core // 2, core % 2
    heads = list(range(hg * HPC, hg * HPC + HPC))
    xt32 = np.ascontiguousarray(x[b].T)
    xt = xt32.astype(BF16)
    xt8 = xt32.astype(E4M3)
    wr = np.asarray(w_qkv, np.float32).reshape(C, 3, H, C)
    w4 = np.ascontiguousarray(
        wr[:, :, heads, :].transpose(0, 2, 1, 3)
    ).reshape(C, 3 * HPC * C)
    wp = np.asarray(w_proj, np.float32).reshape(H, C, C)[heads].reshape(HPC * C, C)
    bias = (np.asarray(b_proj, np.float32) if hg == 0
            else np.zeros(C, np.float32)).reshape(1, C)
    ident = np.eye(128, dtype=np.float32).astype(BF16)
    tri = np.where(np.arange(128)[:, None] > np.arange(128)[None, :],
                   np.float32(-30000.0), np.float32(0.0))
    uneg = np.zeros((128, 2, 256), np.float32)
    uneg[:, 0, 0:128] = tri                 # slab0: tri in first 128 cols
    uneg[:, 1, 0:128] = -30000.0            # slab1: fully masked region
    uneg[:, 1, 128:256] = tri               # slab1: tri in second 128 cols
    uneg = uneg.astype(BF16)
    return {"xt": xt, "xt8": xt8,
            "wqkv": w4.astype(BF16), "wqkv8": w4.astype(E4M3),
            "wproj": wp.astype(BF16), "wproj8": wp.astype(E4M3),
            "bias": bias, "ident": ident, "uneg": uneg}


_CACHE = {}


def kernel(x, w_qkv, w_proj, b_proj, **run_kwargs):
    x = np.asarray(x, np.float32)
    w_qkv = np.asarray(w_qkv, np.float32)
    w_proj = np.asarray(w_proj, np.float32)
    b_proj = np.asarray(b_proj, np.float32)
    if "nc" not in _CACHE:
        _CACHE["nc"] = build_program(N)
    nc = _CACHE["nc"]
    in_maps = [core_inputs(c, x, w_qkv, w_proj, b_proj) for c in range(8)]
    res = run_bass_kernel_spmd(nc, in_maps, core_ids=list(range(8)), **run_kwargs)
    out = np.zeros((B, N, C), np.float32)
    for c in range(8):
        out[c // 2] += res.results[c]["out"]
    _CACHE["last_results"] = res
    return out
